# revision 54
# baseline (speedup 1.0000x reference)
"""Trainium2 Bass kernel for nn_CrossAttentionModel (cross-attention pooling).

Strategy
-------------
Data-parallel over batch: core i handles batch item i (B=8, 8 cores, no
collectives).  Host folds the weight chain and precomputes the tiny
per-sequence H matrices; the device runs the O(NT*H) pair grid, which is
>95% of the FLOPs:

    H1 = e1 @ w1a              (host, [n1,H])
    H2 = e2 @ w1b + tb1        (host, [n2,H])
    rs    = H1[l] + H2[m]                      DVE  (fp16)
    rhv   = relu(rs) * 64                      DVE  (fp16, value path)
    rtb   = fp8(relu(rs) * 64)                 ACT  (one fused pass)
    s     = rtb @ (32*W_a.fp8)                 PE   fp8 DoubleRow (2 col/cyc)
    at    = fp8(128*relu(s + b_a))             ACT
    logit = at @ (32*aw2.fp8)                  PE   fp8 DoubleRow
    attn  = sigmoid(logit/4096 + ab2)          ACT
    attnm = attn * valid                       DVE  (+ TR -> S)
    v     = rhv @ (512*w_c as 2 fp16 limbs)    PE   (limbs sum in PSUM)
    Pw   += sum(attnm * v)                     DVE
    y     = (Pw/(64*512) + S*t_c)/(S+1e-5)+cb

W_a = tw2 @ aw1 folds the trans-MLP second layer into the attn MLP
(emb @ aw1 = rh @ W_a + tb2@aw1), and w_c = tw2 @ cw projects the pooled
value so only the scalar per-pair projection v_p = rh_p . w_c is needed
(pooled @ cw = (sum attn*rh) @ w_c / denom + ...).  The fp8 attention path
is safe because logits are tiny (sigmoid ~ 0.5 + logit/4); the value path
needs w_c kept near-exact (shared quantization noise does not average out
over pairs), hence the two-limb fp16 split of w_c on the PE.

HW notes: tensor_tensor_reduce on single-partition rows crashes the TRN2
exec unit (use TT+TR); DoubleRow lhsT plane strides must be 16B-aligned;
a dummy-matmul chain at program start warms the PE clock gate during the
runtime preamble + input DMAs.
"""

import numpy as np

B, L1, L2, D, HH, V = 8, 64, 64, 768, 1024, 50257
PAD_ID = 50257
P = 128
DC = D // P    # 6 chunks of the 768 attn dims
HC = HH // P   # 8 chunks of the 1024 hidden dims
HP = HC // 2   # 4 DoubleRow h-pair groups
DP = DC // 2   # 3 DoubleRow d-pair groups

SC_RH = 64.0    # rhv / rtb scale
SC_WA = 32.0    # W_a fp8 scale
SC_AT = 128.0   # at fp8 scale
SC_A2 = 32.0    # aw2 fp8 scale
SC_WC = 512.0   # w_c limb scale

_prog_cache = {}


def _build_program(N1, N2P, K, NBLK, ab2_f, cb_f, t_c_f, warm=78):
    import concourse.bass as bass
    import concourse.bacc as bacc
    import concourse.mybir as mybir
    import concourse.tile as tile

    f32 = mybir.dt.float32
    f32r = mybir.dt.float32r
    f16 = mybir.dt.float16
    f8 = mybir.dt.float8e4
    Act = mybir.ActivationFunctionType
    Alu = mybir.AluOpType
    Axis = mybir.AxisListType
    DR = mybir.MatmulPerfMode.DoubleRow

    NP = K * N1                 # pairs per block
    NPR = (NP + 1) & ~1         # fp16 plane stride (4B-aligned slices)
    NPS = (NP + 15) & ~15       # fp8 plane stride (mult of 16)

    # fp16 blob column offsets: h1t | h2t | wcl
    O1 = 0
    O2 = HC * N1
    OW = O2 + HC * N2P
    W16 = OW + 2 * HC
    # fp8 blob column offsets: wa (4 groups of [2, D]) | aw2 (DP slots of 32)
    OA2 = HP * 2 * D
    W8 = OA2 + DP * 32

    nc = bacc.Bacc(
        "TRN2",
        target_bir_lowering=False,
        debug=False,
        enable_asserts=False,
        num_devices=8,
    )

    b16_d = nc.dram_tensor("b16", [P, W16], f16, kind="ExternalInput").ap()
    b8_d = nc.dram_tensor("b8", [P, W8], f8, kind="ExternalInput").ap()
    bac_d = nc.dram_tensor("bac", [P, DC], f32, kind="ExternalInput").ap()
    vld_d = nc.dram_tensor("vld", [1, NBLK * NPS], f32, kind="ExternalInput").ap()
    y_d = nc.dram_tensor("y", [1, 1], f32, kind="ExternalOutput").ap()

    with tile.TileContext(nc, trace_sim=False) as tc:
        with (
            tc.tile_pool(name="const", bufs=1) as cpool,
            tc.tile_pool(name="work", bufs=1) as work,
            tc.tile_pool(name="ps", bufs=4, space="PSUM") as psp,
            tc.tile_pool(name="psv", bufs=2, space="PSUM") as psv,
        ):
            b16 = cpool.tile([P, W16], f16)
            nc.sync.dma_start(b16[:], b16_d[:])
            vld = cpool.tile([1, NBLK * NPS], f32)
            nc.sync.dma_start(vld[:], vld_d[:])
            bac = cpool.tile([P, DC], f32)
            nc.sync.dma_start(bac[:], bac_d[:])
            b8 = cpool.tile([P, W8], f8)
            nc.scalar.dma_start(b8[:], b8_d[:])

            def h1s(hc):
                return b16[:, O1 + hc * N1:O1 + (hc + 1) * N1]

            def h2s(hc, bi):
                o = O2 + hc * N2P + bi * K
                return b16[:, o:o + K]

            def wclp(hc):
                o = OW + 2 * hc
                return b16[:, o:o + 2]

            def wa3(hp):
                return b8[:, hp * 2 * D:(hp + 1) * 2 * D].rearrange(
                    "p (j d) -> p j d", j=2)

            # PE clock-gate warm-up: dummy accumulation chain on scratch data,
            # no input dependencies, runs during the preamble + input DMAs.
            if warm:
                wsc = cpool.tile([P, 64], f16)
                nc.vector.memset(wsc[:], 0.25)
                wps = psv.tile([1, NP], f32, tag="pv", bufs=2, name="warmps")
                for wi in range(warm):
                    nc.tensor.matmul(
                        wps[:, :64], lhsT=wsc[:, :1], rhs=wsc[:],
                        start=(wi == 0), stop=(wi == warm - 1),
                    )

            Pw_parts = work.tile([1, NBLK], f32, tag="pwp", bufs=1)
            S_parts = work.tile([1, NBLK], f32, tag="sp", bufs=1)

            for bi in range(NBLK):
                # rs = H1[l] + H2[m]  (fp16, [P, HC, K*N1])
                rs = work.tile([P, HC, NPR], f16, tag="rs", bufs=3, name=f"rs{bi}")
                rhv = work.tile([P, HC, NPR], f16, tag="rhv", bufs=3, name=f"rhv{bi}")
                for hc in range(HC):
                    nc.vector.tensor_tensor(
                        out=rs[:, hc, :NP].rearrange("p (k l) -> p k l", k=K),
                        in0=h1s(hc).unsqueeze(1).broadcast_to([P, K, N1]),
                        in1=h2s(hc, bi).unsqueeze(2).broadcast_to([P, K, N1]),
                        op=Alu.add,
                    )
                    # rhv right after its rs chunk so the v-matmuls start early
                    nc.vector.tensor_scalar(
                        out=rhv[:, hc, :NP], in0=rs[:, hc, :NP],
                        scalar1=0.0, scalar2=SC_RH,
                        op0=Alu.max, op1=Alu.mult,
                    )
                # rtb = fp8(64*relu(rs)) in DoubleRow layout [P, HC, NPS]
                # (two halves so the first s-matmuls can start earlier)
                rtb = work.tile([P, HC, NPS], f8, tag="rtb", bufs=3, name=f"rtb{bi}")
                hh = HC // 2
                nc.scalar.activation(
                    rtb[:, :hh, :NP], rs[:, :hh, :NP], Act.Relu, scale=SC_RH,
                )
                nc.scalar.activation(
                    rtb[:, hh:, :NP], rs[:, hh:, :NP], Act.Relu, scale=SC_RH,
                )

                def emit_v(bi=bi, rhv=rhv):
                    # v = rhv @ (wcl0 + wcl1) -> pv [1, NP] (limbs accumulate)
                    pv = psv.tile([1, NP], f32, tag="pv", bufs=2,
                                  name=f"pv{bi}")
                    for j in range(2):
                        for hc in range(HC):
                            nc.tensor.matmul(
                                pv[:],
                                lhsT=wclp(hc)[:, j:j + 1],
                                rhs=rhv[:, hc, :NP],
                                start=(j == 0 and hc == 0),
                                stop=(j == 1 and hc == HC - 1),
                            )
                    return pv

                def emit_s_logit(bi=bi, rtb=rtb):
                    # s = rtb @ W_a (fp8 DoubleRow), at = fp8(128*relu(s+b_a))
                    at = work.tile([P, DC, NPS], f8, tag="at", bufs=3,
                                   name=f"at{bi}")
                    for dc in range(DC):
                        ps = psp.tile([P, NP], f32, tag="ps",
                                      name=f"ps{bi}_{dc}")
                        for hp in range(HP):
                            nc.tensor.matmul(
                                ps[:],
                                lhsT=wa3(hp)[:, :, dc * P:(dc + 1) * P],
                                rhs=rtb[:, 2 * hp:2 * hp + 2, :NP],
                                start=(hp == 0),
                                stop=(hp == HP - 1),
                                perf_mode=DR,
                            )
                        nc.scalar.activation(
                            at[:, dc, :NP], ps[:], Act.Relu,
                            bias=bac[:, dc:dc + 1],
                            scale=SC_AT / (SC_RH * SC_WA),
                        )
                    # logit = at @ aw2  ([1, NP] PSUM)
                    pl = psv.tile([1, NP], f32, tag="pl", bufs=2,
                                  name=f"pl{bi}")
                    for dp in range(DP):
                        nc.tensor.matmul(
                            pl[:],
                            lhsT=b8[:, OA2 + dp * 32:OA2 + (dp + 1) * 32]
                                .rearrange("p (j s) -> p j s", j=2)[:, :, 0:1],
                            rhs=at[:, 2 * dp:2 * dp + 2, :NP],
                            start=(dp == 0),
                            stop=(dp == DP - 1),
                            perf_mode=DR,
                        )
                    return pl

                if bi == NBLK - 1:
                    # last block: s-chain first so the sigmoid/pooling tail
                    # overlaps the trailing v-matmuls on the PE
                    pl = emit_s_logit()
                    pv = emit_v()
                else:
                    pv = emit_v()
                    pl = emit_s_logit()
                # attn = sigmoid(logit/4096 + ab2)
                attn = work.tile([1, NP], f32, tag="attn", bufs=2, name=f"attn{bi}")
                nc.scalar.activation(
                    attn[:], pl[:], Act.Sigmoid,
                    bias=float(ab2_f), scale=1.0 / (SC_AT * SC_A2),
                )
                # attnm = attn * valid; S_parts[bi] = sum(attnm)
                attnm = work.tile([1, NP], f32, tag="attnm", bufs=2, name=f"attnm{bi}")
                nc.vector.tensor_tensor(
                    out=attnm[:], in0=attn[:],
                    in1=vld[:, bi * NPS:bi * NPS + NP], op=Alu.mult,
                )
                nc.vector.tensor_reduce(
                    out=S_parts[:, bi:bi + 1], in_=attnm[:],
                    axis=Axis.X, op=Alu.add,
                )
                # Pw_parts[bi] = sum(attnm * (v0 + v1))
                vm = work.tile([1, NP], f32, tag="vm", bufs=2, name=f"vm{bi}")
                nc.vector.tensor_tensor(
                    out=vm[:], in0=attnm[:], in1=pv[0:1, :], op=Alu.mult,
                )
                nc.vector.tensor_reduce(
                    out=Pw_parts[:, bi:bi + 1], in_=vm[:],
                    axis=Axis.X, op=Alu.add,
                )

            # ---- final: y = (Pw/(64*512) + S*t_c)/(S+1e-5) + cb ----
            Pw = work.tile([1, 1], f32, tag="pw", bufs=1)
            nc.vector.tensor_reduce(out=Pw[:], in_=Pw_parts[:], axis=Axis.X,
                                    op=Alu.add)
            S = work.tile([1, 1], f32, tag="s", bufs=1)
            nc.vector.tensor_reduce(out=S[:], in_=S_parts[:], axis=Axis.X,
                                    op=Alu.add)
            den = work.tile([1, 1], f32, tag="den", bufs=1)
            nc.vector.tensor_scalar_add(den[:], S[:], 1e-5)
            rden = work.tile([1, 1], f32, tag="rden", bufs=1)
            nc.vector.reciprocal(rden[:], den[:])
            num = work.tile([1, 1], f32, tag="num", bufs=1)
            pw_sc = work.tile([1, 1], f32, tag="pwsc", bufs=1)
            nc.vector.tensor_scalar_mul(pw_sc[:], Pw[:], 1.0 / (SC_RH * SC_WC))
            nc.vector.scalar_tensor_tensor(
                out=num[:], in0=S[:], scalar=float(t_c_f), in1=pw_sc[:],
                op0=Alu.mult, op1=Alu.add,
            )
            y0 = work.tile([1, 1], f32, tag="y0", bufs=1)
            nc.vector.tensor_tensor(out=y0[:], in0=num[:], in1=rden[:],
                                    op=Alu.mult)
            y1 = work.tile([1, 1], f32, tag="y1", bufs=1)
            nc.vector.tensor_scalar_add(y1[:], y0[:], float(cb_f))
            nc.sync.dma_start(y_d[:], y1[:])

    nc.compile()
    return nc


def _prep(x1, x2, mask1, mask2, embed_table, tw1, tb1, tw2, tb2,
          aw1, ab1, aw2, ab2, cw, cb, compact=True):
    """Host-side prep: compaction, weight folding, H matmuls, per-core maps."""
    import ml_dtypes
    f32 = np.float32
    f16 = np.float16
    f8 = ml_dtypes.float8_e4m3fn
    f64 = np.float64

    x1 = np.where(x1 == PAD_ID, 0, x1).astype(np.int32)
    x2 = np.where(x2 == PAD_ID, 0, x2).astype(np.int32)
    w1a = np.ascontiguousarray(tw1[:D]).astype(f64)
    w1b = np.ascontiguousarray(tw1[D:]).astype(f64)
    W_a = (tw2.astype(f64) @ aw1.astype(f64)).astype(f32)
    b_a = (tb2.astype(f64) @ aw1.astype(f64) + ab1.astype(f64)).astype(f32)
    w_c = (tw2.astype(f64) @ cw.astype(f64)).astype(f32).ravel()
    t_c = float(tb2.astype(f64) @ cw.astype(f64).ravel())

    if compact:
        l_lists = [np.nonzero(mask1[b])[0] for b in range(B)]
        m_lists = [np.nonzero(mask2[b])[0] for b in range(B)]
        N1 = max(4, max((len(l) for l in l_lists), default=4))
        N2 = max(1, max((len(m) for m in m_lists), default=1))
    else:
        l_lists = [np.arange(L1) for _ in range(B)]
        m_lists = [np.arange(L2) for _ in range(B)]
        N1, N2 = L1, L2
    K_max = max(1, min(16, 512 // N1))
    NBLK = -(-N2 // K_max)
    K = -(-N2 // NBLK)          # shrink K to just cover N2 in NBLK blocks
    N2P = NBLK * K
    NP = K * N1
    NPS = (NP + 15) & ~15

    O1 = 0
    O2 = HC * N1
    OW = O2 + HC * N2P
    W16 = OW + 2 * HC
    OA2 = HP * 2 * D
    W8 = OA2 + DP * 32

    # fp8 blob: wa groups + aw2 (shared across cores)
    b8_host = np.zeros((P, W8), dtype=f8)
    for hp in range(HP):
        for j in range(2):
            b8_host[:, hp * 2 * D + j * D:(hp * 2 + j + 1) * D] = \
                (SC_WA * W_a[(2 * hp + j) * P:(2 * hp + j + 1) * P, :]).astype(f8)
    a2 = (SC_A2 * aw2.astype(f32).ravel()).reshape(DC, P)  # [dc, p]
    for dp in range(DP):
        for j in range(2):
            b8_host[:, OA2 + dp * 32 + j * 16] = a2[2 * dp + j, :].astype(f8)

    bac_host = np.ascontiguousarray((SC_AT * b_a).reshape(DC, P).T, dtype=f32)
    wc_s = (SC_WC * w_c).astype(f32)
    l0 = wc_s.astype(f16)
    l1 = (wc_s - l0.astype(f32)).astype(f16)

    table = np.asarray(embed_table, dtype=f32)
    in_maps = []
    for b in range(B):
        ll, ml = l_lists[b], m_lists[b]
        n1, n2 = len(ll), len(ml)
        b16_host = np.zeros((P, W16), dtype=f16)
        b16_host[:, OW + 0:OW + 2 * HC:2] = l0.reshape(HC, P).T
        b16_host[:, OW + 1:OW + 2 * HC:2] = l1.reshape(HC, P).T
        if n1:
            e1 = table[x1[b][ll]].astype(f64)          # [n1, D]
            H1 = (e1 @ w1a).astype(f32)                # [n1, HH]
            h1 = H1.T.reshape(HC, P, n1)               # [hc, p, l]
            b16_host[:, O1:O2].reshape(P, HC, N1)[:, :, :n1] = \
                np.transpose(h1, (1, 0, 2)).astype(f16)
        if n2:
            e2 = table[x2[b][ml]].astype(f64)
            H2 = (e2 @ w1b + tb1.astype(f64)).astype(f32)
            h2 = H2.T.reshape(HC, P, n2)
            b16_host[:, O2:OW].reshape(P, HC, N2P)[:, :, :n2] = \
                np.transpose(h2, (1, 0, 2)).astype(f16)
        vld = np.zeros((NBLK, NPS), dtype=f32)
        if n1 and n2:
            vm = (mask1[b][ll][None, :] != 0) & (mask2[b][ml][:, None] != 0) \
                 & (x1[b][ll][None, :] != x2[b][ml][:, None])   # [n2, n1]
            grid = np.zeros((N2P, N1), dtype=f32)
            grid[:n2, :n1] = vm.astype(f32)
            vld[:, :NP] = grid.reshape(NBLK, K * N1)
        in_maps.append({
            "b16": b16_host,
            "b8": b8_host,
            "bac": bac_host,
            "vld": vld.reshape(1, NBLK * NPS),
        })
    ab2_f = float(np.asarray(ab2).ravel()[0])
    cb_f = float(np.asarray(cb).ravel()[0])
    return (N1, N2P, K, NBLK, ab2_f, cb_f, t_c), in_maps


def kernel(x1, x2, mask1, mask2, embed_table, tw1, tb1, tw2, tb2,
           aw1, ab1, aw2, ab2, cw, cb):
    from concourse import bass_utils

    key_args, in_maps = _prep(
        x1, x2, mask1, mask2, embed_table, tw1, tb1, tw2, tb2,
        aw1, ab1, aw2, ab2, cw, cb)

    if key_args not in _prog_cache:
        _prog_cache[key_args] = _build_program(*key_args)
    nc = _prog_cache[key_args]

    res = bass_utils.run_bass_kernel_spmd(nc, in_maps, core_ids=list(range(8)))
    y = np.stack([res.results[i]["y"].reshape(()) for i in range(B)])
    return y.reshape(B, 1).astype(np.float32)


# revision 55
# speedup vs baseline: 1.0446x; 1.0446x over previous
"""Trainium2 Bass kernel for nn_CrossAttentionModel (cross-attention pooling).

Strategy
-------------
Data-parallel over batch: core i handles batch item i (B=8, 8 cores, no
collectives).  Host folds the weight chain and precomputes the tiny
per-sequence H matrices; the device runs the O(NT*H) pair grid, which is
>95% of the FLOPs:

    H1 = e1 @ w1a              (host, [n1,H])
    H2 = e2 @ w1b + tb1        (host, [n2,H])
    rs    = H1[l] + H2[m]                      DVE  (fp16)
    rhv   = relu(rs) * 64                      DVE  (fp16, value path)
    rtb   = fp8(relu(rs) * 64)                 ACT  (one fused pass)
    s     = rtb @ (32*W_a.fp8)                 PE   fp8 DoubleRow (2 col/cyc)
    at    = fp8(128*relu(s + b_a))             ACT
    logit = at @ (32*aw2.fp8)                  PE   fp8 DoubleRow
    attn  = sigmoid(logit/4096 + ab2)          ACT
    attnm = attn * valid                       DVE  (+ TR -> S)
    v     = rhv @ (512*w_c as 2 fp16 limbs)    PE   (limbs sum in PSUM)
    Pw   += sum(attnm * v)                     DVE
    y     = (Pw/(64*512) + S*t_c)/(S+1e-5)+cb

W_a = tw2 @ aw1 folds the trans-MLP second layer into the attn MLP
(emb @ aw1 = rh @ W_a + tb2@aw1), and w_c = tw2 @ cw projects the pooled
value so only the scalar per-pair projection v_p = rh_p . w_c is needed
(pooled @ cw = (sum attn*rh) @ w_c / denom + ...).  The fp8 attention path
is safe because logits are tiny (sigmoid ~ 0.5 + logit/4); the value path
needs w_c kept near-exact (shared quantization noise does not average out
over pairs), hence the two-limb fp16 split of w_c on the PE.

HW notes: tensor_tensor_reduce on single-partition rows crashes the TRN2
exec unit (use TT+TR); DoubleRow lhsT plane strides must be 16B-aligned;
a dummy-matmul chain at program start warms the PE clock gate during the
runtime preamble + input DMAs.
"""

import numpy as np

B, L1, L2, D, HH, V = 8, 64, 64, 768, 1024, 50257
PAD_ID = 50257
P = 128
DC = D // P    # 6 chunks of the 768 attn dims
HC = HH // P   # 8 chunks of the 1024 hidden dims
HP = HC // 2   # 4 DoubleRow h-pair groups
DP = DC // 2   # 3 DoubleRow d-pair groups

SC_RH = 64.0    # rhv / rtb scale
SC_WA = 32.0    # W_a fp8 scale
SC_AT = 128.0   # at fp8 scale
SC_A2 = 32.0    # aw2 fp8 scale
SC_WC = 512.0   # w_c limb scale

_prog_cache = {}


def _build_program(N1, N2P, K, NBLK, ab2_f, cb_f, t_c_f, warm=85):
    import concourse.bass as bass
    import concourse.bacc as bacc
    import concourse.mybir as mybir
    import concourse.tile as tile

    f32 = mybir.dt.float32
    f32r = mybir.dt.float32r
    f16 = mybir.dt.float16
    f8 = mybir.dt.float8e4
    Act = mybir.ActivationFunctionType
    Alu = mybir.AluOpType
    Axis = mybir.AxisListType
    DR = mybir.MatmulPerfMode.DoubleRow

    NP = K * N1                 # pairs per block
    NPR = (NP + 1) & ~1         # fp16 plane stride (4B-aligned slices)
    NPS = (NP + 15) & ~15       # fp8 plane stride (mult of 16)

    # fp16 blob column offsets: h1t | h2t | wcl
    O1 = 0
    O2 = HC * N1
    OW = O2 + HC * N2P
    W16 = OW + 2 * HC
    # fp8 blob column offsets: wa (4 groups of [2, D]) | aw2 (DP slots of 32)
    OA2 = HP * 2 * D
    W8 = OA2 + DP * 32

    nc = bacc.Bacc(
        "TRN2",
        target_bir_lowering=False,
        debug=False,
        enable_asserts=False,
        num_devices=8,
    )

    b16_d = nc.dram_tensor("b16", [P, W16], f16, kind="ExternalInput").ap()
    b8_d = nc.dram_tensor("b8", [P, W8], f8, kind="ExternalInput").ap()
    bac_d = nc.dram_tensor("bac", [P, DC], f32, kind="ExternalInput").ap()
    vld_d = nc.dram_tensor("vld", [1, NBLK * NPS], f32, kind="ExternalInput").ap()
    y_d = nc.dram_tensor("y", [1, 1], f32, kind="ExternalOutput").ap()

    with tile.TileContext(nc, trace_sim=False) as tc:
        with (
            tc.tile_pool(name="const", bufs=1) as cpool,
            tc.tile_pool(name="work", bufs=1) as work,
            tc.tile_pool(name="ps", bufs=4, space="PSUM") as psp,
            tc.tile_pool(name="psv", bufs=2, space="PSUM") as psv,
        ):
            b16 = cpool.tile([P, W16], f16)
            nc.sync.dma_start(b16[:], b16_d[:])
            vld = cpool.tile([1, NBLK * NPS], f32)
            nc.sync.dma_start(vld[:], vld_d[:])
            bac = cpool.tile([P, DC], f32)
            nc.sync.dma_start(bac[:], bac_d[:])
            b8 = cpool.tile([P, W8], f8)
            nc.scalar.dma_start(b8[:], b8_d[:])

            def h1s(hc):
                return b16[:, O1 + hc * N1:O1 + (hc + 1) * N1]

            def h2s(hc, bi):
                o = O2 + hc * N2P + bi * K
                return b16[:, o:o + K]

            def wclp(hc):
                o = OW + 2 * hc
                return b16[:, o:o + 2]

            def wa3(hp):
                return b8[:, hp * 2 * D:(hp + 1) * 2 * D].rearrange(
                    "p (j d) -> p j d", j=2)

            # PE clock-gate warm-up: dummy accumulation chain on scratch data,
            # no input dependencies, runs during the preamble + input DMAs.
            if warm:
                wsc = cpool.tile([P, 64], f16)
                nc.vector.memset(wsc[:], 0.25)
                wps = psv.tile([1, NP], f32, tag="pv", bufs=2, name="warmps")
                for wi in range(warm):
                    nc.tensor.matmul(
                        wps[:, :64], lhsT=wsc[:, :1], rhs=wsc[:],
                        start=(wi == 0), stop=(wi == warm - 1),
                    )

            Pw_parts = work.tile([1, NBLK], f32, tag="pwp", bufs=1)
            S_parts = work.tile([1, NBLK], f32, tag="sp", bufs=1)

            for bi in range(NBLK):
                # rs = H1[l] + H2[m]  (fp16, [P, HC, K*N1])
                rs = work.tile([P, HC, NPR], f16, tag="rs", bufs=3, name=f"rs{bi}")
                rhv = work.tile([P, HC, NPR], f16, tag="rhv", bufs=3, name=f"rhv{bi}")
                for hc in range(HC):
                    nc.vector.tensor_tensor(
                        out=rs[:, hc, :NP].rearrange("p (k l) -> p k l", k=K),
                        in0=h1s(hc).unsqueeze(1).broadcast_to([P, K, N1]),
                        in1=h2s(hc, bi).unsqueeze(2).broadcast_to([P, K, N1]),
                        op=Alu.add,
                    )
                    # rhv right after its rs chunk so the v-matmuls start early
                    nc.vector.tensor_scalar(
                        out=rhv[:, hc, :NP], in0=rs[:, hc, :NP],
                        scalar1=0.0, scalar2=SC_RH,
                        op0=Alu.max, op1=Alu.mult,
                    )
                # rtb = fp8(64*relu(rs)) in DoubleRow layout [P, HC, NPS]
                # (two halves so the first s-matmuls can start earlier)
                rtb = work.tile([P, HC, NPS], f8, tag="rtb", bufs=3, name=f"rtb{bi}")
                hh = HC // 2
                nc.scalar.activation(
                    rtb[:, :hh, :NP], rs[:, :hh, :NP], Act.Relu, scale=SC_RH,
                )
                nc.scalar.activation(
                    rtb[:, hh:, :NP], rs[:, hh:, :NP], Act.Relu, scale=SC_RH,
                )

                def emit_v(bi=bi, rhv=rhv):
                    # v = rhv @ (wcl0 + wcl1) -> pv [1, NP] (limbs accumulate)
                    pv = psv.tile([1, NP], f32, tag="pv", bufs=2,
                                  name=f"pv{bi}")
                    for j in range(2):
                        for hc in range(HC):
                            nc.tensor.matmul(
                                pv[:],
                                lhsT=wclp(hc)[:, j:j + 1],
                                rhs=rhv[:, hc, :NP],
                                start=(j == 0 and hc == 0),
                                stop=(j == 1 and hc == HC - 1),
                            )
                    return pv

                def emit_s_logit(bi=bi, rtb=rtb):
                    # s = rtb @ W_a (fp8 DoubleRow), at = fp8(128*relu(s+b_a))
                    at = work.tile([P, DC, NPS], f8, tag="at", bufs=2,
                                   name=f"at{bi}")
                    for dc in range(DC):
                        ps = psp.tile([P, NP], f32, tag="ps",
                                      name=f"ps{bi}_{dc}")
                        for hp in range(HP):
                            nc.tensor.matmul(
                                ps[:],
                                lhsT=wa3(hp)[:, :, dc * P:(dc + 1) * P],
                                rhs=rtb[:, 2 * hp:2 * hp + 2, :NP],
                                start=(hp == 0),
                                stop=(hp == HP - 1),
                                perf_mode=DR,
                            )
                        nc.scalar.activation(
                            at[:, dc, :NP], ps[:], Act.Relu,
                            bias=bac[:, dc:dc + 1],
                            scale=SC_AT / (SC_RH * SC_WA),
                        )
                    # logit = at @ aw2  ([1, NP] PSUM)
                    pl = psv.tile([1, NP], f32, tag="pl", bufs=2,
                                  name=f"pl{bi}")
                    for dp in range(DP):
                        nc.tensor.matmul(
                            pl[:],
                            lhsT=b8[:, OA2 + dp * 32:OA2 + (dp + 1) * 32]
                                .rearrange("p (j s) -> p j s", j=2)[:, :, 0:1],
                            rhs=at[:, 2 * dp:2 * dp + 2, :NP],
                            start=(dp == 0),
                            stop=(dp == DP - 1),
                            perf_mode=DR,
                        )
                    return pl

                if bi == NBLK - 1:
                    # last block: s-chain first so the sigmoid/pooling tail
                    # overlaps the trailing v-matmuls on the PE
                    pl = emit_s_logit()
                    pv = emit_v()
                else:
                    pv = emit_v()
                    pl = emit_s_logit()
                # attn = sigmoid(logit/4096 + ab2)
                attn = work.tile([1, NP], f32, tag="attn", bufs=2, name=f"attn{bi}")
                nc.scalar.activation(
                    attn[:], pl[:], Act.Sigmoid,
                    bias=float(ab2_f), scale=1.0 / (SC_AT * SC_A2),
                )
                # attnm = attn * valid; S_parts[bi] = sum(attnm)
                attnm = work.tile([1, NP], f32, tag="attnm", bufs=2, name=f"attnm{bi}")
                nc.vector.tensor_tensor(
                    out=attnm[:], in0=attn[:],
                    in1=vld[:, bi * NPS:bi * NPS + NP], op=Alu.mult,
                )
                nc.vector.tensor_reduce(
                    out=S_parts[:, bi:bi + 1], in_=attnm[:],
                    axis=Axis.X, op=Alu.add,
                )
                # Pw_parts[bi] = sum(attnm * (v0 + v1))
                vm = work.tile([1, NP], f32, tag="vm", bufs=2, name=f"vm{bi}")
                nc.vector.tensor_tensor(
                    out=vm[:], in0=attnm[:], in1=pv[0:1, :], op=Alu.mult,
                )
                nc.vector.tensor_reduce(
                    out=Pw_parts[:, bi:bi + 1], in_=vm[:],
                    axis=Axis.X, op=Alu.add,
                )

            # ---- final: y = (Pw/(64*512) + S*t_c)/(S+1e-5) + cb ----
            Pw = work.tile([1, 1], f32, tag="pw", bufs=1)
            nc.vector.tensor_reduce(out=Pw[:], in_=Pw_parts[:], axis=Axis.X,
                                    op=Alu.add)
            S = work.tile([1, 1], f32, tag="s", bufs=1)
            nc.vector.tensor_reduce(out=S[:], in_=S_parts[:], axis=Axis.X,
                                    op=Alu.add)
            den = work.tile([1, 1], f32, tag="den", bufs=1)
            nc.vector.tensor_scalar_add(den[:], S[:], 1e-5)
            rden = work.tile([1, 1], f32, tag="rden", bufs=1)
            nc.vector.reciprocal(rden[:], den[:])
            num = work.tile([1, 1], f32, tag="num", bufs=1)
            pw_sc = work.tile([1, 1], f32, tag="pwsc", bufs=1)
            nc.vector.tensor_scalar_mul(pw_sc[:], Pw[:], 1.0 / (SC_RH * SC_WC))
            nc.vector.scalar_tensor_tensor(
                out=num[:], in0=S[:], scalar=float(t_c_f), in1=pw_sc[:],
                op0=Alu.mult, op1=Alu.add,
            )
            y0 = work.tile([1, 1], f32, tag="y0", bufs=1)
            nc.vector.tensor_tensor(out=y0[:], in0=num[:], in1=rden[:],
                                    op=Alu.mult)
            y1 = work.tile([1, 1], f32, tag="y1", bufs=1)
            nc.vector.tensor_scalar_add(y1[:], y0[:], float(cb_f))
            nc.sync.dma_start(y_d[:], y1[:])

    nc.compile()
    return nc


def _prep(x1, x2, mask1, mask2, embed_table, tw1, tb1, tw2, tb2,
          aw1, ab1, aw2, ab2, cw, cb, compact=True):
    """Host-side prep: compaction, weight folding, H matmuls, per-core maps."""
    import ml_dtypes
    f32 = np.float32
    f16 = np.float16
    f8 = ml_dtypes.float8_e4m3fn
    f64 = np.float64

    x1 = np.where(x1 == PAD_ID, 0, x1).astype(np.int32)
    x2 = np.where(x2 == PAD_ID, 0, x2).astype(np.int32)
    w1a = np.ascontiguousarray(tw1[:D]).astype(f64)
    w1b = np.ascontiguousarray(tw1[D:]).astype(f64)
    W_a = (tw2.astype(f64) @ aw1.astype(f64)).astype(f32)
    b_a = (tb2.astype(f64) @ aw1.astype(f64) + ab1.astype(f64)).astype(f32)
    w_c = (tw2.astype(f64) @ cw.astype(f64)).astype(f32).ravel()
    t_c = float(tb2.astype(f64) @ cw.astype(f64).ravel())

    if compact:
        l_lists = [np.nonzero(mask1[b])[0] for b in range(B)]
        m_lists = [np.nonzero(mask2[b])[0] for b in range(B)]
        N1 = max(4, max((len(l) for l in l_lists), default=4))
        N2 = max(1, max((len(m) for m in m_lists), default=1))
    else:
        l_lists = [np.arange(L1) for _ in range(B)]
        m_lists = [np.arange(L2) for _ in range(B)]
        N1, N2 = L1, L2
    K_max = max(1, min(16, 512 // N1))
    NBLK = -(-N2 // K_max)
    K = -(-N2 // NBLK)          # shrink K to just cover N2 in NBLK blocks
    N2P = NBLK * K
    NP = K * N1
    NPS = (NP + 15) & ~15

    O1 = 0
    O2 = HC * N1
    OW = O2 + HC * N2P
    W16 = OW + 2 * HC
    OA2 = HP * 2 * D
    W8 = OA2 + DP * 32

    # fp8 blob: wa groups + aw2 (shared across cores)
    b8_host = np.zeros((P, W8), dtype=f8)
    for hp in range(HP):
        for j in range(2):
            b8_host[:, hp * 2 * D + j * D:(hp * 2 + j + 1) * D] = \
                (SC_WA * W_a[(2 * hp + j) * P:(2 * hp + j + 1) * P, :]).astype(f8)
    a2 = (SC_A2 * aw2.astype(f32).ravel()).reshape(DC, P)  # [dc, p]
    for dp in range(DP):
        for j in range(2):
            b8_host[:, OA2 + dp * 32 + j * 16] = a2[2 * dp + j, :].astype(f8)

    bac_host = np.ascontiguousarray((SC_AT * b_a).reshape(DC, P).T, dtype=f32)
    wc_s = (SC_WC * w_c).astype(f32)
    l0 = wc_s.astype(f16)
    l1 = (wc_s - l0.astype(f32)).astype(f16)

    table = np.asarray(embed_table, dtype=f32)
    in_maps = []
    for b in range(B):
        ll, ml = l_lists[b], m_lists[b]
        n1, n2 = len(ll), len(ml)
        b16_host = np.zeros((P, W16), dtype=f16)
        b16_host[:, OW + 0:OW + 2 * HC:2] = l0.reshape(HC, P).T
        b16_host[:, OW + 1:OW + 2 * HC:2] = l1.reshape(HC, P).T
        if n1:
            e1 = table[x1[b][ll]].astype(f64)          # [n1, D]
            H1 = (e1 @ w1a).astype(f32)                # [n1, HH]
            h1 = H1.T.reshape(HC, P, n1)               # [hc, p, l]
            b16_host[:, O1:O2].reshape(P, HC, N1)[:, :, :n1] = \
                np.transpose(h1, (1, 0, 2)).astype(f16)
        if n2:
            e2 = table[x2[b][ml]].astype(f64)
            H2 = (e2 @ w1b + tb1.astype(f64)).astype(f32)
            h2 = H2.T.reshape(HC, P, n2)
            b16_host[:, O2:OW].reshape(P, HC, N2P)[:, :, :n2] = \
                np.transpose(h2, (1, 0, 2)).astype(f16)
        vld = np.zeros((NBLK, NPS), dtype=f32)
        if n1 and n2:
            vm = (mask1[b][ll][None, :] != 0) & (mask2[b][ml][:, None] != 0) \
                 & (x1[b][ll][None, :] != x2[b][ml][:, None])   # [n2, n1]
            grid = np.zeros((N2P, N1), dtype=f32)
            grid[:n2, :n1] = vm.astype(f32)
            vld[:, :NP] = grid.reshape(NBLK, K * N1)
        in_maps.append({
            "b16": b16_host,
            "b8": b8_host,
            "bac": bac_host,
            "vld": vld.reshape(1, NBLK * NPS),
        })
    ab2_f = float(np.asarray(ab2).ravel()[0])
    cb_f = float(np.asarray(cb).ravel()[0])
    return (N1, N2P, K, NBLK, ab2_f, cb_f, t_c), in_maps


def kernel(x1, x2, mask1, mask2, embed_table, tw1, tb1, tw2, tb2,
           aw1, ab1, aw2, ab2, cw, cb):
    from concourse import bass_utils

    key_args, in_maps = _prep(
        x1, x2, mask1, mask2, embed_table, tw1, tb1, tw2, tb2,
        aw1, ab1, aw2, ab2, cw, cb)

    if key_args not in _prog_cache:
        _prog_cache[key_args] = _build_program(*key_args)
    nc = _prog_cache[key_args]

    res = bass_utils.run_bass_kernel_spmd(nc, in_maps, core_ids=list(range(8)))
    y = np.stack([res.results[i]["y"].reshape(()) for i in range(B)])
    return y.reshape(B, 1).astype(np.float32)
